# revision 9
# baseline (speedup 1.0000x reference)
"""DeepSeek-MLA attention Trainium2 Bass kernel, 8-core SPMD.

Sharding (one NEFF, per-core data differs):
  - Tokens (B*S = 4096) sharded 512/core for down-projections and o_proj.
  - Heads (16) sharded 2/core for up-projections and attention.
  - Collectives stitch the shardings:
      AllGather(kv_norm^T + k_rope^T)  after the joint kv down-proj (bf16),
      AllGather(q_lora_raw^T) x3 + AllGather(1/rms) after the q down-proj,
      AllToAll(attn_out^T) x2          head-parallel -> token-parallel (bf16).
  - All heavy matmuls run in bf16 (same PE rate as f32r at N>=256 but half
    the DMA/SBUF bytes); PSUM accumulation is always f32.
  - hidden^T is pre-transposed on the host (O(n^2) reshuffle), so the kernel
    has no PE transposes.
  - Causal softmax without running max (scores are O(+-7) post-scale);
    masking is a multiplicative 0/1 bf16 mask applied on the Vector engine
    after exp; the denominator l = sum_k P accumulates on the Vector engine
    (pacc += pt per k-block) with a single ones-column matmul per q-chunk.
  - Wo is fully preloaded into SBUF during the attention phase so o_proj
    never waits on weight DMA.

RMSNorm weights are folded into the up-projection weights on the host.
"""

import math

import numpy as np

# ---- problem shapes (hardcoded; harness contract) ----
B, S, HID = 2, 2048, 2048
IN = 2 * HID
H = 16
NOPE, ROPE, VHD = 128, 64, 128
QKD = NOPE + ROPE
QR, KVR = 1536, 512
EPS = 1e-6
THETA = 10000.0
SCALE = 1.0 / math.sqrt(QKD)

NCORES = 8
T = B * S                 # 4096 flat tokens (b-major)
TSH = T // NCORES         # 512 tokens per core
HPC = H // NCORES         # 2 heads per core

_cache = {}


def _build():
    import concourse.bass as bass
    import concourse.mybir as mybir
    import concourse.tile as tile
    from concourse import bacc

    dt = mybir.dt
    F32 = dt.float32
    F32R = dt.float32r
    BF16 = dt.bfloat16
    AF = mybir.ActivationFunctionType

    nc = bacc.Bacc("TRN2", target_bir_lowering=False, debug=False,
                   num_devices=NCORES)

    # ---------------- I/O ----------------
    def inp(name, shape, dtype=BF16):
        return nc.dram_tensor(name, shape, dtype, kind="ExternalInput").ap()

    hidt = inp("hidt", [IN // 128, 128, TSH])         # pre-transposed shard
    wqa = inp("wqa", [IN // 128, 128, QR])            # full
    wqb = inp("wqb", [QR // 128, 128, HPC * QKD])     # shard, cols reordered
    wkva = inp("wkva", [IN // 128, 128, KVR + ROPE])  # full
    wkvb_kn = inp("wkvb_kn", [KVR // 128, 128, HPC * NOPE])
    wkvb_v = inp("wkvb_v", [KVR // 128, 128, HPC * VHD])
    wo = inp("wo", [H * VHD // 128, 128, HID])        # full
    cos_k = inp("cos_k", [ROPE, TSH], dt.float32)
    sin_k = inp("sin_k", [ROPE, TSH], dt.float32)
    cos_q = inp("cos_q", [2 * ROPE, T], dt.float32)   # doubled for 2 heads
    sin_q = inp("sin_q", [2 * ROPE, T], dt.float32)
    mask01 = inp("mask01", [4, 128, 512])             # bf16 0/1 causal mask
    r128 = inp("r128", [128, 128], F32R)              # q-rope rotation lhsT
    r64 = inp("r64", [ROPE, ROPE], F32R)              # k-rope rotation lhsT
    onesc = inp("onesc", [128, 1], F32R)
    onesr = inp("onesr", [1, 128], F32R)

    y = nc.dram_tensor("y", [TSH, HID], F32, kind="ExternalOutput").ap()

    QRC = QR // 128            # 12 q-lora chunks
    KVC = KVR // 128           # 4 kv chunks
    INC = IN // 128            # 32 input chunks
    TC = T // 512              # 8 token chunks (flat)
    SB = S // 512              # 4 token chunks per batch
    NKB = S // 128             # 16 key blocks per batch
    KCO = H * VHD // 128       # 16 o_proj contraction chunks

    with tile.TileContext(nc) as tc:
        with tc.tile_pool(name="dram", bufs=1, space="DRAM") as dram, \
             tc.tile_pool(name="const", bufs=1) as const:

            # ---- DRAM bounce buffers for collectives ----
            ag_kv_in = dram.tile([KVR + ROPE, TSH], BF16)
            ag_kv_out = dram.tile([NCORES, KVR + ROPE, TSH], BF16,
                                  addr_space="Shared")
            ag_q_in = [dram.tile([QR // 3, TSH], BF16, name=f"ag_q_in{g}")
                       for g in range(3)]
            ag_q_out = [dram.tile([NCORES, QR // 3, TSH], BF16,
                                  addr_space="Shared", name=f"ag_q_out{g}")
                        for g in range(3)]
            ag_iv_in = dram.tile([1, TSH], F32R)
            ag_iv_out = dram.tile([NCORES, 1, TSH], F32R, addr_space="Shared")
            a2a_in = [dram.tile([NCORES, VHD, TSH], BF16, name=f"a2a_in{hl}")
                      for hl in range(HPC)]
            a2a_out = [dram.tile([NCORES, VHD, TSH], BF16, name=f"a2a_out{hl}")
                       for hl in range(HPC)]

            # ---- small constants resident in SBUF ----
            r128_sb = const.tile([128, 128], F32R)
            nc.sync.dma_start(r128_sb[:], r128[:])
            r64_sb = const.tile([ROPE, ROPE], F32R)
            nc.sync.dma_start(r64_sb[:], r64[:])
            onesc_sb = const.tile([128, 1], F32R)
            nc.sync.dma_start(onesc_sb[:], onesc[:])
            onesr_sb = const.tile([1, 128], F32R)
            nc.sync.dma_start(onesr_sb[:], onesr[:])
            cosk_sb = const.tile([ROPE, TSH], F32)
            nc.sync.dma_start(cosk_sb[:], cos_k[:])
            sink_sb = const.tile([ROPE, TSH], F32)
            nc.sync.dma_start(sink_sb[:], sin_k[:])
            mask_sb = const.tile([128, 4, 512], BF16)
            for r in range(4):
                nc.sync.dma_start(mask_sb[:, r, :], mask01[r])
            eps_sb = const.tile([1, 1], F32)
            nc.vector.memset(eps_sb[:], EPS)

            # ============ phase B: down-projections + AllGathers ==========
            with tc.tile_pool(name="b_sbuf", bufs=1) as bpool, \
                 tc.tile_pool(name="b_w", bufs=6) as bw, \
                 tc.tile_pool(name="b_stage", bufs=3) as bst:

                hidT = bpool.tile([128, INC, TSH], BF16)   # 32 KB/part
                for k in range(INC):
                    nc.sync.dma_start(hidT[:, k, :], hidt[k])

                with tc.tile_pool(name="dp_ps", bufs=5, space="PSUM") as dp_ps, \
                     tc.tile_pool(name="ss_ps", bufs=1, space="PSUM") as ss_ps, \
                     tc.tile_pool(name="ms_ps", bufs=1, space="PSUM") as ms_ps:

                    sumsq_q = ss_ps.tile([1, TSH], F32, tag="ssq")

                    def q_group(g):
                        q_ps = [dp_ps.tile([128, TSH], F32, name=f"qps{m}",
                                           tag="dps") for m in range(4)]
                        for k in range(INC):
                            wt = bw.tile([128, 512], BF16, name="wqa_t",
                                         tag="wqa_t")
                            nc.scalar.dma_start(
                                wt[:], wqa[k, :, g * 512:(g + 1) * 512])
                            for m in range(4):
                                nc.tensor.matmul(
                                    q_ps[m][:], wt[:, m * 128:(m + 1) * 128],
                                    hidT[:, k, :],
                                    start=(k == 0), stop=(k == INC - 1))
                        for m in range(4):
                            mg = g * 4 + m
                            qr_t = bst.tile([128, TSH], BF16, name="qr_t",
                                            tag="qr_t")
                            nc.scalar.copy(qr_t[:], q_ps[m][:])
                            nc.sync.dma_start(
                                ag_q_in[g][m * 128:(m + 1) * 128, :], qr_t[:])
                            sq = bst.tile([128, TSH], F32R, name="sq", tag="sq")
                            nc.vector.tensor_mul(sq[:], qr_t[:], qr_t[:])
                            nc.tensor.matmul(sumsq_q[:], onesc_sb[:], sq[:],
                                             start=(mg == 0),
                                             stop=(mg == QRC - 1))
                        if g == 2:
                            # inv-rms AG goes just before the last q AG
                            s_q = bst.tile([1, TSH], F32, tag="s_small")
                            nc.scalar.activation(s_q[:], sumsq_q[:], AF.Sqrt,
                                                 bias=eps_sb[:], scale=1.0 / QR)
                            inv_q = bst.tile([1, TSH], F32R, tag="inv_small")
                            with nc.allow_low_precision("f32r intended"):
                                nc.vector.reciprocal(inv_q[:], s_q[:])
                            nc.sync.dma_start(ag_iv_in[:], inv_q[:])
                            nc.gpsimd.collective_compute(
                                "AllGather", mybir.AluOpType.bypass,
                                replica_groups=[list(range(NCORES))],
                                ins=[ag_iv_in.opt()], outs=[ag_iv_out.opt()])
                        nc.gpsimd.collective_compute(
                            "AllGather", mybir.AluOpType.bypass,
                            replica_groups=[list(range(NCORES))],
                            ins=[ag_q_in[g].opt()], outs=[ag_q_out[g].opt()])

                    def kv_down():
                        kv_ps = [dp_ps.tile([128, TSH], F32, name=f"kvps{m}",
                                            tag="dps") for m in range(KVC)]
                        kr_ps = dp_ps.tile([ROPE, TSH], F32, tag="dps")
                        for k in range(INC):
                            wt = bw.tile([128, KVR + ROPE], BF16, name="wkva_t",
                                         tag="wkva_t")
                            nc.scalar.dma_start(wt[:], wkva[k])
                            for m in range(KVC):
                                nc.tensor.matmul(
                                    kv_ps[m][:], wt[:, m * 128:(m + 1) * 128],
                                    hidT[:, k, :],
                                    start=(k == 0), stop=(k == INC - 1))
                            nc.tensor.matmul(
                                kr_ps[:], wt[:, KVR:], hidT[:, k, :],
                                start=(k == 0), stop=(k == INC - 1))

                        # rms over kv chunks
                        kv_raw = [bpool.tile([128, TSH], F32, name=f"kvraw{m}",
                                             tag=f"kvraw{m}")
                                  for m in range(KVC)]
                        sumsq_kv = ss_ps.tile([1, TSH], F32, tag="ssq2")
                        for m in range(KVC):
                            nc.scalar.copy(kv_raw[m][:], kv_ps[m][:])
                            sq = bst.tile([128, TSH], F32R, name="sqk",
                                          tag="sq")
                            nc.vector.tensor_mul(sq[:], kv_raw[m][:],
                                                 kv_raw[m][:])
                            nc.tensor.matmul(sumsq_kv[:], onesc_sb[:], sq[:],
                                             start=(m == 0),
                                             stop=(m == KVC - 1))
                        s_kv = bst.tile([1, TSH], F32, tag="s_small")
                        nc.scalar.activation(s_kv[:], sumsq_kv[:], AF.Sqrt,
                                             bias=eps_sb[:], scale=1.0 / KVR)
                        inv_kv = bst.tile([1, TSH], F32R, tag="inv_small")
                        with nc.allow_low_precision("f32r rounding intended"):
                            nc.vector.reciprocal(inv_kv[:], s_kv[:])
                        binv = ms_ps.tile([128, TSH], F32, tag="msps")
                        nc.tensor.matmul(binv[:], onesr_sb[:], inv_kv[:],
                                         start=True, stop=True)
                        for m in range(KVC):
                            kvn = bst.tile([128, TSH], BF16, name="kvn",
                                           tag="kvn")
                            nc.vector.tensor_mul(kvn[:], kv_raw[m][:], binv[:])
                            nc.sync.dma_start(
                                ag_kv_in[m * 128:(m + 1) * 128, :], kvn[:])

                        # k-rope: rotate + cos/sin (token shard only)
                        krope_raw = bst.tile([ROPE, TSH], F32R, tag="krr")
                        nc.scalar.copy(krope_raw[:], kr_ps[:])
                        rot_ps = ms_ps.tile([ROPE, TSH], F32, tag="msps")
                        nc.tensor.matmul(rot_ps[:], r64_sb[:], krope_raw[:],
                                         start=True, stop=True)
                        t1 = bst.tile([ROPE, TSH], F32, tag="ropet1")
                        nc.vector.tensor_mul(t1[:], krope_raw[:], cosk_sb[:])
                        t2 = bst.tile([ROPE, TSH], F32, tag="ropet2")
                        nc.vector.tensor_mul(t2[:], rot_ps[:], sink_sb[:])
                        krn = bst.tile([ROPE, TSH], BF16, tag="krn")
                        nc.vector.tensor_add(krn[:], t1[:], t2[:])
                        nc.sync.dma_start(ag_kv_in[KVR:, :], krn[:])
                        nc.gpsimd.collective_compute(
                            "AllGather", mybir.AluOpType.bypass,
                            replica_groups=[list(range(NCORES))],
                            ins=[ag_kv_in.opt()], outs=[ag_kv_out.opt()])

                    # CC-stream friendly order: q0, q1, kv, q2
                    q_group(0)
                    q_group(1)
                    kv_down()
                    q_group(2)

            # ============ phase C: up-projections (head-parallel) ==========
            with tc.tile_pool(name="kn_sb", bufs=1) as kn_pool, \
                 tc.tile_pool(name="v_sb", bufs=1) as v_pool, \
                 tc.tile_pool(name="qt_sb", bufs=1) as qt_pool, \
                 tc.tile_pool(name="kr_sb", bufs=1) as kr_pool, \
                 tc.tile_pool(name="wo_sb", bufs=1) as wo_pool:

                knT = kn_pool.tile([128, HPC, TC, 512], BF16)   # 16 KB/part
                v_sb = v_pool.tile([128, TSH // 128 * NCORES, HPC * VHD], BF16)
                qT = qt_pool.tile([128, 3, TC, 512], BF16)      # 24 KB/part
                # k_rope^T doubled into both partition halves so that the
                # rope score matmul's lhsT base_partition matches q's half
                krT = kr_pool.tile([2 * ROPE, TC, 512], BF16)
                nc.sync.dma_start(
                    krT[0:ROPE, :, :],
                    ag_kv_out[:, KVR:, :].transpose([1, 0, 2]))
                nc.sync.dma_start(
                    krT[ROPE:, :, :],
                    ag_kv_out[:, KVR:, :].transpose([1, 0, 2]))
                # Wo preload, head-0 half only (finishes during attention;
                # 32 KB/part bf16). Odd (head-1) chunks stream in phase E.
                wo_sbuf = wo_pool.tile([128, KCO // 2, HID], BF16)

                with tc.tile_pool(name="kvn_sb", bufs=8) as kvn_pool, \
                     tc.tile_pool(name="upw", bufs=1) as upw, \
                     tc.tile_pool(name="up_ps", bufs=6, space="PSUM") as up_ps:
                    wkn_sb = upw.tile([128, KVC, HPC * NOPE], BF16)
                    for k in range(KVC):
                        nc.scalar.dma_start(wkn_sb[:, k, :], wkvb_kn[k])
                    wv_sb = upw.tile([128, KVC, HPC * VHD], BF16)
                    for k in range(KVC):
                        nc.scalar.dma_start(wv_sb[:, k, :], wkvb_v[k])

                    # K_nope^T and V, streaming kv_norm^T tiles from the AG
                    for tcb in range(TC):
                        rh = []
                        for k in range(KVC):
                            rt = kvn_pool.tile([128, 512], BF16, name="kvn_t",
                                               tag="kvn_t")
                            nc.sync.dma_start(
                                rt[:],
                                ag_kv_out[tcb, k * 128:(k + 1) * 128, :])
                            rh.append(rt)
                        psn = [up_ps.tile([128, 512], F32, name=f"knps{hl}",
                                          tag="upps") for hl in range(HPC)]
                        for k in range(KVC):
                            for hl in range(HPC):
                                nc.tensor.matmul(
                                    psn[hl][:],
                                    wkn_sb[:, k, hl * NOPE:(hl + 1) * NOPE],
                                    rh[k][:],
                                    start=(k == 0), stop=(k == KVC - 1))
                        for hl in range(HPC):
                            nc.scalar.copy(knT[:, hl, tcb, :], psn[hl][:])
                        psv = [up_ps.tile([128, HPC * VHD], F32,
                                          name=f"vps{j}", tag="upps")
                               for j in range(4)]
                        for k in range(KVC):
                            for j in range(4):
                                nc.tensor.matmul(
                                    psv[j][:],
                                    rh[k][:, j * 128:(j + 1) * 128],
                                    wv_sb[:, k, :],
                                    start=(k == 0), stop=(k == KVC - 1))
                        for j in range(4):
                            nc.vector.tensor_copy(v_sb[:, tcb * 4 + j, :],
                                                  psv[j][:])

                # Q^T (3 chunks: nope h0, nope h1, rope doubled), deferred
                # RMS normalize folded into the PSUM eviction, rope applied
                # per token-chunk right after.
                with tc.tile_pool(name="agq_sb", bufs=8) as agq_pool, \
                     tc.tile_pool(name="qw_sb", bufs=1) as qw_pool, \
                     tc.tile_pool(name="rope_sb", bufs=2) as rope_pool, \
                     tc.tile_pool(name="ropest", bufs=2) as ropest, \
                     tc.tile_pool(name="qt_ps", bufs=4, space="PSUM") as qt_ps, \
                     tc.tile_pool(name="rr_ps", bufs=2, space="PSUM") as rr_ps, \
                     tc.tile_pool(name="bq_ps", bufs=2, space="PSUM") as bq_ps:
                    wqb_sb = qw_pool.tile([128, QRC, HPC * QKD], BF16)
                    for k in range(QRC):
                        nc.scalar.dma_start(wqb_sb[:, k, :], wqb[k])
                    invq_sb = qw_pool.tile([1, TC, 512], F32R)
                    nc.sync.dma_start(
                        invq_sb[:],
                        ag_iv_out.rearrange("r o t -> o r t"))
                    for tcb in range(TC):
                        ps = [qt_ps.tile([128, 512], F32, name=f"qtps{m}",
                                         tag="qtps") for m in range(3)]
                        for k in range(QRC):
                            rh16 = agq_pool.tile([128, 512], BF16, name="agq16",
                                                 tag="agq16")
                            nc.sync.dma_start(
                                rh16[:],
                                ag_q_out[k // 4][tcb,
                                                 (k % 4) * 128:(k % 4 + 1) * 128,
                                                 :])
                            for m in range(3):
                                nc.tensor.matmul(
                                    ps[m][:],
                                    wqb_sb[:, k, m * 128:(m + 1) * 128],
                                    rh16[:],
                                    start=(k == 0), stop=(k == QRC - 1))
                        # broadcast 1/rms across partitions, then evict with
                        # the normalize multiply
                        biq = bq_ps.tile([128, 512], F32, name="biq", tag="biq")
                        nc.tensor.matmul(biq[:], onesr_sb[:],
                                         invq_sb[:, tcb, :],
                                         start=True, stop=True)
                        biq_sb = ropest.tile([128, 512], F32, name="biq_sb",
                                             tag="biq_sb")
                        nc.scalar.copy(biq_sb[:], biq[:])
                        for m in range(2):
                            nc.vector.tensor_mul(qT[:, m, tcb, :], ps[m][:],
                                                 biq_sb[:])
                        # q-rope on chunk m=2 (both heads doubled)
                        qr2 = ropest.tile([128, 512], F32R, name="qr2",
                                          tag="qr2")
                        nc.vector.tensor_mul(qr2[:], ps[2][:], biq_sb[:])
                        cosq_t = rope_pool.tile([128, 512], F32, name="cosq_t",
                                                tag="cosq_t")
                        nc.sync.dma_start(cosq_t[:],
                                          cos_q[:, tcb * 512:(tcb + 1) * 512])
                        sinq_t = rope_pool.tile([128, 512], F32, name="sinq_t",
                                                tag="sinq_t")
                        nc.sync.dma_start(sinq_t[:],
                                          sin_q[:, tcb * 512:(tcb + 1) * 512])
                        rps = rr_ps.tile([128, 512], F32, tag="rrps")
                        nc.tensor.matmul(rps[:], r128_sb[:], qr2[:],
                                         start=True, stop=True)
                        t1 = ropest.tile([128, 512], F32, name="rt1", tag="rt1")
                        nc.vector.tensor_mul(t1[:], qr2[:], cosq_t[:])
                        t2 = ropest.tile([128, 512], F32, name="rt2", tag="rt2")
                        nc.vector.tensor_mul(t2[:], rps[:], sinq_t[:])
                        nc.vector.tensor_add(qT[:, 2, tcb, :], t1[:], t2[:])

                # ============ phase D: causal attention =================
                with tc.tile_pool(name="ao_sb", bufs=1) as ao_pool:
                  aosb = ao_pool.tile([128, KCO, 512], BF16)
                  # kick off the Wo even-kc preload now; lands during attention
                  for i in range(KCO // 2):
                      nc.sync.dma_start(wo_sbuf[:, i, :], wo[2 * i])
                  with tc.tile_pool(name="pt_sb", bufs=6) as pt_pool, \
                     tc.tile_pool(name="pa_sb", bufs=2) as pa_pool, \
                     tc.tile_pool(name="att_st", bufs=2) as att_st, \
                     tc.tile_pool(name="st_ps", bufs=3, space="PSUM") as st_ps, \
                     tc.tile_pool(name="l_ps", bufs=2, space="PSUM") as l_ps, \
                     tc.tile_pool(name="o_ps", bufs=2, space="PSUM") as o_ps, \
                     tc.tile_pool(name="bi_ps", bufs=1, space="PSUM") as bi_ps:
                    for hl in range(HPC):
                        for b in range(B):
                            for qc in range(SB):
                                tcq = b * SB + qc
                                nkb = 4 * qc + 4
                                op = o_ps.tile([128, 512], F32, name="op",
                                               tag="op")
                                pacc = pa_pool.tile([128, 512], F32R,
                                                    name="pacc", tag="pacc")
                                for kb in range(nkb):
                                    tck = b * SB + kb // 4
                                    co = (kb % 4) * 128
                                    st = st_ps.tile([128, 512], F32,
                                                    name="st", tag="st")
                                    nc.tensor.matmul(
                                        st[:],
                                        knT[:, hl, tck, co:co + 128],
                                        qT[:, hl, tcq, :],
                                        start=True, stop=False)
                                    nc.tensor.matmul(
                                        st[:],
                                        krT[hl * ROPE:(hl + 1) * ROPE,
                                            tck, co:co + 128],
                                        qT[hl * ROPE:(hl + 1) * ROPE, 2, tcq, :],
                                        start=False, stop=True)
                                    diag = kb >= 4 * qc
                                    pt = pt_pool.tile([128, 512], BF16,
                                                      name="pt", tag="pt")
                                    if diag:
                                        pte = pt_pool.tile([128, 512], BF16,
                                                           name="pte",
                                                           tag="pte", bufs=2)
                                        nc.scalar.activation(pte[:], st[:],
                                                             AF.Exp,
                                                             scale=SCALE)
                                        nc.vector.tensor_mul(
                                            pt[:], pte[:],
                                            mask_sb[:, kb - 4 * qc, :])
                                    else:
                                        nc.scalar.activation(pt[:], st[:],
                                                             AF.Exp,
                                                             scale=SCALE)
                                    if kb == 0:
                                        nc.vector.tensor_copy(pacc[:], pt[:])
                                    else:
                                        nc.vector.tensor_add(pacc[:], pacc[:],
                                                             pt[:])
                                    nc.tensor.matmul(
                                        op[:],
                                        v_sb[:, b * NKB + kb,
                                             hl * VHD:(hl + 1) * VHD],
                                        pt[:],
                                        start=(kb == 0), stop=(kb == nkb - 1))
                                lp = l_ps.tile([1, 512], F32, name="lp",
                                               tag="lp")
                                nc.tensor.matmul(lp[:], onesc_sb[:], pacc[:],
                                                 start=True, stop=True)
                                invl = att_st.tile([1, 512], F32R, name="invl",
                                                   tag="invl")
                                with nc.allow_low_precision("f32r intended"):
                                    nc.vector.reciprocal(invl[:], lp[:])
                                bi = bi_ps.tile([128, 512], F32, name="bi",
                                                tag="bi")
                                nc.tensor.matmul(bi[:], onesr_sb[:], invl[:],
                                                 start=True, stop=True)
                                ot = att_st.tile([128, 512], F32, name="ot",
                                                 tag="ot")
                                nc.scalar.copy(ot[:], op[:])
                                att = att_st.tile([128, 512], BF16, name="att",
                                                  tag="att")
                                nc.vector.tensor_mul(att[:], ot[:], bi[:])
                                nc.sync.dma_start(
                                    a2a_in[hl][tcq, :, :], att[:])
                        nc.gpsimd.collective_compute(
                            "AllToAll", mybir.AluOpType.bypass,
                            replica_groups=[list(range(NCORES))],
                            ins=[a2a_in[hl].opt()], outs=[a2a_out[hl].opt()])
                        for i in range(NCORES):
                            nc.sync.dma_start(aosb[:, 2 * i + hl, :],
                                              a2a_out[hl][i])

                  # ============ phase E: o_proj (token-parallel) ===========
                  with tc.tile_pool(name="yo_sb", bufs=3) as yo_pool, \
                       tc.tile_pool(name="wo_st", bufs=6) as wo_st, \
                       tc.tile_pool(name="op_ps", bufs=8, space="PSUM") as op_ps:
                    # accumulate kc chunks head-0-first (preloaded in SBUF) so
                    # o_proj can start while the second AllToAll is in flight;
                    # head-1 (odd kc) weights stream from DRAM.
                    kc_order = [2 * i for i in range(NCORES)] + \
                               [2 * i + 1 for i in range(NCORES)]
                    for n in range(HID // 512):
                        pso = [op_ps.tile([128, 512], F32, name=f"pso{mt}",
                                          tag="pso") for mt in range(4)]
                        for ki, kc in enumerate(kc_order):
                            if kc % 2 == 0:
                                wot = wo_sbuf[:, kc // 2,
                                              n * 512:(n + 1) * 512]
                            else:
                                wot_t = wo_st.tile([128, 512], BF16,
                                                   name="wot", tag="wot")
                                nc.scalar.dma_start(
                                    wot_t[:], wo[kc, :, n * 512:(n + 1) * 512])
                                wot = wot_t[:]
                            for mt in range(4):
                                nc.tensor.matmul(
                                    pso[mt][:],
                                    aosb[:, kc, mt * 128:(mt + 1) * 128],
                                    wot,
                                    start=(ki == 0), stop=(ki == KCO - 1))
                        for mt in range(4):
                            yt = yo_pool.tile([128, 512], F32, name="yt",
                                              tag="yt")
                            nc.scalar.copy(yt[:], pso[mt][:])
                            nc.sync.dma_start(
                                y[mt * 128:(mt + 1) * 128,
                                  n * 512:(n + 1) * 512], yt[:])

    nc.compile()
    return nc


def _rot_lhsT(n):
    """lhsT for the interleaved rotate-half as a matmul: out = R @ x,
    R[2i, 2i+1] = -1, R[2i+1, 2i] = +1; matmul computes lhsT.T @ rhs."""
    R = np.zeros((n, n), dtype=np.float32)
    for i in range(n // 2):
        R[2 * i, 2 * i + 1] = -1.0
        R[2 * i + 1, 2 * i] = 1.0
    return np.ascontiguousarray(R.T)


def _prep_inputs(inputs):
    """Host-side sharding/reordering. Returns in_maps (list of 8 dicts)."""
    import ml_dtypes
    BF = ml_dtypes.bfloat16

    hs = np.asarray(inputs["hidden_states"], dtype=np.float32).reshape(T, IN)
    hsT = np.ascontiguousarray(hs.T)                  # [IN, T] host transpose
    Wq_a = np.asarray(inputs["Wq_a"], dtype=np.float32)
    q_a_ln = np.asarray(inputs["q_a_ln"], dtype=np.float32)
    Wq_b = np.asarray(inputs["Wq_b"], dtype=np.float32) * q_a_ln[:, None]
    Wkv_a = np.asarray(inputs["Wkv_a"], dtype=np.float32)
    kv_a_ln = np.asarray(inputs["kv_a_ln"], dtype=np.float32)
    Wkv_b = np.asarray(inputs["Wkv_b"], dtype=np.float32) * kv_a_ln[:, None]
    Wo = np.asarray(inputs["Wo"], dtype=np.float32)
    pos = np.asarray(inputs["position_ids"]).astype(np.float64)   # [B, S]

    # rope tables (doubled pairs): cos[2i] = cos[2i+1] = cos(pos * invf_i)
    invf = 1.0 / (THETA ** (np.arange(0, ROPE, 2, dtype=np.float64) / ROPE))
    fr = pos[..., None] * invf                       # [B, S, 32]
    cosd = np.repeat(np.cos(fr), 2, axis=-1).astype(np.float32)  # [B, S, 64]
    sind = np.repeat(np.sin(fr), 2, axis=-1).astype(np.float32)
    cosT = np.ascontiguousarray(cosd.reshape(T, ROPE).T)   # [64, T]
    sinT = np.ascontiguousarray(sind.reshape(T, ROPE).T)
    cos_q = np.concatenate([cosT, cosT], axis=0)           # [128, T]
    sin_q = np.concatenate([sinT, sinT], axis=0)

    # causal multiplicative 0/1 mask for diagonal blocks
    mask01 = np.zeros((4, 128, 512), dtype=BF)
    kl = np.arange(128)[:, None]
    ql = np.arange(512)[None, :]
    for r in range(4):
        mask01[r] = np.where(kl + 128 * r > ql, 0.0, 1.0).astype(BF)

    onesc = np.ones((128, 1), dtype=np.float32)
    onesr = np.ones((1, 128), dtype=np.float32)

    wqa_r = np.ascontiguousarray(
        Wq_a.reshape(IN // 128, 128, QR).astype(BF))
    wkva_r = np.ascontiguousarray(
        Wkv_a.reshape(IN // 128, 128, KVR + ROPE).astype(BF))
    wo_r = np.ascontiguousarray(
        Wo.reshape(H * VHD // 128, 128, HID).astype(BF))

    Wq_b_h = Wq_b.reshape(QR, H, QKD)
    Wkv_b_h = Wkv_b.reshape(KVR, H, NOPE + VHD)

    in_maps = []
    for c in range(NCORES):
        h0, h1 = HPC * c, HPC * c + 1
        bc = c // (NCORES // B)
        s0 = (c % (NCORES // B)) * TSH
        # reorder q_b cols: [nope_h0 | nope_h1 | rope_h0 ; rope_h1]
        wqb_s = np.concatenate([
            Wq_b_h[:, h0, :NOPE], Wq_b_h[:, h1, :NOPE],
            Wq_b_h[:, h0, NOPE:], Wq_b_h[:, h1, NOPE:]], axis=1)
        wqb_s = np.ascontiguousarray(
            wqb_s.reshape(QR // 128, 128, HPC * QKD).astype(BF))
        wkvb_kn_s = np.ascontiguousarray(
            np.concatenate([Wkv_b_h[:, h0, :NOPE], Wkv_b_h[:, h1, :NOPE]],
                           axis=1).reshape(KVR // 128, 128,
                                           HPC * NOPE).astype(BF))
        wkvb_v_s = np.ascontiguousarray(
            np.concatenate([Wkv_b_h[:, h0, NOPE:], Wkv_b_h[:, h1, NOPE:]],
                           axis=1).reshape(KVR // 128, 128,
                                           HPC * VHD).astype(BF))
        tok0 = c * TSH
        hidt_s = np.ascontiguousarray(
            hsT[:, tok0:tok0 + TSH].reshape(IN // 128, 128, TSH).astype(BF))
        in_maps.append({
            "hidt": hidt_s,
            "wqa": wqa_r, "wqb": wqb_s, "wkva": wkva_r,
            "wkvb_kn": wkvb_kn_s, "wkvb_v": wkvb_v_s, "wo": wo_r,
            "cos_k": np.ascontiguousarray(
                cosT[:, bc * S + s0: bc * S + s0 + TSH]),
            "sin_k": np.ascontiguousarray(
                sinT[:, bc * S + s0: bc * S + s0 + TSH]),
            "cos_q": cos_q, "sin_q": sin_q,
            "mask01": mask01,
            "r128": np.ascontiguousarray(
                np.block([[_rot_lhsT(ROPE), np.zeros((ROPE, ROPE), np.float32)],
                          [np.zeros((ROPE, ROPE), np.float32), _rot_lhsT(ROPE)]])),
            "r64": _rot_lhsT(ROPE),
            "onesc": onesc, "onesr": onesr,
        })
    return in_maps


def kernel(**inputs) -> np.ndarray:
    from concourse.bass_utils import run_bass_kernel_spmd

    if "nc" not in _cache:
        _cache["nc"] = _build()
    nc = _cache["nc"]
    in_maps = _prep_inputs(inputs)
    res = run_bass_kernel_spmd(nc, in_maps, core_ids=list(range(NCORES)))
    out = np.concatenate([res.results[c]["y"] for c in range(NCORES)], axis=0)
    return np.ascontiguousarray(out.reshape(B, S, HID))


if __name__ == "__main__":
    rng = np.random.default_rng(0)
    ins = {
        "hidden_states": rng.standard_normal((B, S, IN), dtype=np.float32),
        "Wq_a": rng.standard_normal((IN, QR), dtype=np.float32) * IN ** -0.5,
        "q_a_ln": np.ones(QR, np.float32),
        "Wq_b": rng.standard_normal((QR, H * QKD), dtype=np.float32) * QR ** -0.5,
        "Wkv_a": rng.standard_normal((IN, KVR + ROPE), dtype=np.float32) * IN ** -0.5,
        "kv_a_ln": np.ones(KVR, np.float32),
        "Wkv_b": rng.standard_normal((KVR, H * (NOPE + VHD)), dtype=np.float32) * KVR ** -0.5,
        "Wo": rng.standard_normal((H * VHD, HID), dtype=np.float32) * (H * VHD) ** -0.5,
        "position_ids": np.tile(np.arange(S, dtype=np.int32)[None], (B, 1)),
    }
    out = kernel(**ins)
    print("kernel ran, out shape", out.shape, "absmax", np.abs(out).max())


# revision 18
# speedup vs baseline: 1.0938x; 1.0938x over previous
"""DeepSeek-MLA attention Trainium2 Bass kernel, 8-core SPMD.

Sharding (one NEFF, per-core data differs):
  - Tokens (B*S = 4096) sharded 512/core for down-projections and o_proj.
  - Heads (16) sharded 2/core for up-projections and attention.
  - Collectives stitch the shardings:
      AllGather(kv_norm^T + k_rope^T)  after the joint kv down-proj (bf16),
      AllGather(q_lora_raw^T) x3 + AllGather(1/rms) after the q down-proj,
      AllToAll(attn_out^T) x2          head-parallel -> token-parallel (bf16).
  - All heavy matmuls run in bf16 (same PE rate as f32r at N>=256 but half
    the DMA/SBUF bytes); PSUM accumulation is always f32.
  - hidden^T is pre-transposed on the host (O(n^2) reshuffle), so the kernel
    has no PE transposes.
  - Causal softmax without running max (scores are O(+-7) post-scale);
    masking is a multiplicative 0/1 bf16 mask applied on the Vector engine
    after exp; the denominator l = sum_k P accumulates on the Vector engine
    (pacc += pt per k-block) with a single ones-column matmul per q-chunk.
  - Wo is fully preloaded into SBUF during the attention phase so o_proj
    never waits on weight DMA.

RMSNorm weights are folded into the up-projection weights on the host.
"""

import math

import numpy as np

# ---- problem shapes (hardcoded; harness contract) ----
B, S, HID = 2, 2048, 2048
IN = 2 * HID
H = 16
NOPE, ROPE, VHD = 128, 64, 128
QKD = NOPE + ROPE
QR, KVR = 1536, 512
EPS = 1e-6
THETA = 10000.0
SCALE = 1.0 / math.sqrt(QKD)

NCORES = 8
T = B * S                 # 4096 flat tokens (b-major)
TSH = T // NCORES         # 512 tokens per core
HPC = H // NCORES         # 2 heads per core

_cache = {}


def _build():
    import concourse.bass as bass
    import concourse.mybir as mybir
    import concourse.tile as tile
    from concourse import bacc

    dt = mybir.dt
    F32 = dt.float32
    F32R = dt.float32r
    BF16 = dt.bfloat16
    AF = mybir.ActivationFunctionType

    nc = bacc.Bacc("TRN2", target_bir_lowering=False, debug=False,
                   num_devices=NCORES)

    # ---------------- I/O ----------------
    def inp(name, shape, dtype=BF16):
        return nc.dram_tensor(name, shape, dtype, kind="ExternalInput").ap()

    hidt = inp("hidt", [IN // 128, 128, TSH])         # pre-transposed shard
    wqa = inp("wqa", [IN // 128, 128, QR])            # full
    wqb = inp("wqb", [QR // 128, 128, HPC * QKD])     # shard, cols reordered
    wkva = inp("wkva", [IN // 128, 128, KVR + ROPE])  # full
    wkvb_kn = inp("wkvb_kn", [KVR // 128, 128, HPC * NOPE])
    wkvb_v = inp("wkvb_v", [KVR // 128, 128, HPC * VHD])
    wo = inp("wo", [H * VHD // 128, 128, HID])        # full
    cos_k = inp("cos_k", [ROPE, TSH], dt.float32)
    sin_k = inp("sin_k", [ROPE, TSH], dt.float32)
    cos_q = inp("cos_q", [2 * ROPE, T], dt.float32)   # doubled for 2 heads
    sin_q = inp("sin_q", [2 * ROPE, T], dt.float32)
    mask01 = inp("mask01", [128, 128])                # bf16 0/1 diag triangle
    r128 = inp("r128", [128, 128], F32R)              # q-rope rotation lhsT
    r64 = inp("r64", [ROPE, ROPE], F32R)              # k-rope rotation lhsT
    onesc = inp("onesc", [128, 1], F32R)
    onesr = inp("onesr", [1, 128], F32R)
    onesrb = inp("onesrb", [1, 128], BF16)

    y = nc.dram_tensor("y", [TSH, HID], F32, kind="ExternalOutput").ap()

    QRC = QR // 128            # 12 q-lora chunks
    KVC = KVR // 128           # 4 kv chunks
    INC = IN // 128            # 32 input chunks
    TC = T // 512              # 8 token chunks (flat)
    SB = S // 512              # 4 token chunks per batch
    NKB = S // 128             # 16 key blocks per batch
    KCO = H * VHD // 128       # 16 o_proj contraction chunks

    with tile.TileContext(nc) as tc:
        with tc.tile_pool(name="dram", bufs=1, space="DRAM") as dram, \
             tc.tile_pool(name="const", bufs=1) as const:

            # ---- DRAM bounce buffers for collectives ----
            ag_kv_in = dram.tile([KVR + ROPE, TSH], BF16)
            ag_kv_out = dram.tile([NCORES, KVR + ROPE, TSH], BF16,
                                  addr_space="Shared")
            # group 2 carries two extra bf16 rows: hi/lo split of 1/rms(q)
            # (AllGather is a byte copy, so the split reconstructs exactly)
            qg_rows = [QR // 3, QR // 3, QR // 3 + 2]
            ag_q_in = [dram.tile([qg_rows[g], TSH], BF16, name=f"ag_q_in{g}")
                       for g in range(3)]
            ag_q_out = [dram.tile([NCORES, qg_rows[g], TSH], BF16,
                                  addr_space="Shared", name=f"ag_q_out{g}")
                        for g in range(3)]
            a2a_in = [dram.tile([NCORES, VHD, TSH], BF16, name=f"a2a_in{hl}")
                      for hl in range(HPC)]
            a2a_out = [dram.tile([NCORES, VHD, TSH], BF16, name=f"a2a_out{hl}")
                       for hl in range(HPC)]

            # ---- small constants resident in SBUF ----
            r128_sb = const.tile([128, 128], F32R)
            nc.sync.dma_start(r128_sb[:], r128[:])
            r64_sb = const.tile([ROPE, ROPE], F32R)
            nc.sync.dma_start(r64_sb[:], r64[:])
            onesc_sb = const.tile([128, 1], F32R)
            nc.sync.dma_start(onesc_sb[:], onesc[:])
            onesr_sb = const.tile([1, 128], F32R)
            nc.sync.dma_start(onesr_sb[:], onesr[:])
            cosk_sb = const.tile([ROPE, TSH], F32)
            nc.sync.dma_start(cosk_sb[:], cos_k[:])
            sink_sb = const.tile([ROPE, TSH], F32)
            nc.sync.dma_start(sink_sb[:], sin_k[:])
            mask_sb = const.tile([128, 128], BF16)
            nc.sync.dma_start(mask_sb[:], mask01[:])
            onesr_b = const.tile([1, 128], BF16)
            nc.sync.dma_start(onesr_b[:], onesrb[:])
            eps_sb = const.tile([1, 1], F32)
            nc.vector.memset(eps_sb[:], EPS)

            # ============ phase B: down-projections + AllGathers ==========
            with tc.tile_pool(name="b_sbuf", bufs=1) as bpool, \
                 tc.tile_pool(name="b_w", bufs=6) as bw, \
                 tc.tile_pool(name="b_stage", bufs=3) as bst:

                hidT = bpool.tile([128, INC, TSH], BF16)   # 32 KB/part
                for k in range(INC):
                    nc.sync.dma_start(hidT[:, k, :], hidt[k])

                with tc.tile_pool(name="dp_ps", bufs=5, space="PSUM") as dp_ps, \
                     tc.tile_pool(name="ss_ps", bufs=1, space="PSUM") as ss_ps, \
                     tc.tile_pool(name="ms_ps", bufs=1, space="PSUM") as ms_ps:

                    sumsq_q = ss_ps.tile([1, TSH], F32, tag="ssq")

                    def q_group(g):
                        q_ps = [dp_ps.tile([128, TSH], F32, name=f"qps{m}",
                                           tag="dps") for m in range(4)]
                        for k in range(INC):
                            wt = bw.tile([128, 512], BF16, name="wqa_t",
                                         tag="wqa_t")
                            nc.scalar.dma_start(
                                wt[:], wqa[k, :, g * 512:(g + 1) * 512])
                            for m in range(4):
                                nc.tensor.matmul(
                                    q_ps[m][:], wt[:, m * 128:(m + 1) * 128],
                                    hidT[:, k, :],
                                    start=(k == 0), stop=(k == INC - 1))
                        for m in range(4):
                            mg = g * 4 + m
                            qr_t = bst.tile([128, TSH], BF16, name="qr_t",
                                            tag="qr_t")
                            nc.scalar.copy(qr_t[:], q_ps[m][:])
                            nc.sync.dma_start(
                                ag_q_in[g][m * 128:(m + 1) * 128, :], qr_t[:])
                            sq = bst.tile([128, TSH], F32R, name="sq", tag="sq")
                            nc.vector.tensor_mul(sq[:], qr_t[:], qr_t[:])
                            nc.tensor.matmul(sumsq_q[:], onesc_sb[:], sq[:],
                                             start=(mg == 0),
                                             stop=(mg == QRC - 1))
                        if g == 2:
                            # 1/rms rides the g2 AG as two bf16 rows (hi/lo)
                            s_q = bst.tile([1, TSH], F32, tag="s_small")
                            nc.scalar.activation(s_q[:], sumsq_q[:], AF.Sqrt,
                                                 bias=eps_sb[:], scale=1.0 / QR)
                            inv_q = bst.tile([1, TSH], F32R, tag="inv_small")
                            with nc.allow_low_precision("f32r intended"):
                                nc.vector.reciprocal(inv_q[:], s_q[:])
                            iv_hi = bst.tile([1, TSH], BF16, tag="iv_hi")
                            nc.vector.tensor_copy(iv_hi[:], inv_q[:])
                            iv_lo32 = bst.tile([1, TSH], F32, tag="iv_lo32")
                            nc.vector.tensor_sub(iv_lo32[:], inv_q[:],
                                                 iv_hi[:])
                            iv_lo = bst.tile([1, TSH], BF16, tag="iv_lo")
                            nc.vector.tensor_copy(iv_lo[:], iv_lo32[:])
                            nc.sync.dma_start(
                                ag_q_in[2][QR // 3:QR // 3 + 1, :], iv_hi[:])
                            nc.sync.dma_start(
                                ag_q_in[2][QR // 3 + 1:, :], iv_lo[:])
                        nc.gpsimd.collective_compute(
                            "AllGather", mybir.AluOpType.bypass,
                            replica_groups=[list(range(NCORES))],
                            ins=[ag_q_in[g].opt()], outs=[ag_q_out[g].opt()])

                    def kv_down():
                        kv_ps = [dp_ps.tile([128, TSH], F32, name=f"kvps{m}",
                                            tag="dps") for m in range(KVC)]
                        kr_ps = dp_ps.tile([ROPE, TSH], F32, tag="dps")
                        for k in range(INC):
                            wt = bw.tile([128, KVR + ROPE], BF16, name="wkva_t",
                                         tag="wkva_t")
                            nc.scalar.dma_start(wt[:], wkva[k])
                            for m in range(KVC):
                                nc.tensor.matmul(
                                    kv_ps[m][:], wt[:, m * 128:(m + 1) * 128],
                                    hidT[:, k, :],
                                    start=(k == 0), stop=(k == INC - 1))
                            nc.tensor.matmul(
                                kr_ps[:], wt[:, KVR:], hidT[:, k, :],
                                start=(k == 0), stop=(k == INC - 1))

                        # rms over kv chunks
                        kv_raw = [bpool.tile([128, TSH], F32, name=f"kvraw{m}",
                                             tag=f"kvraw{m}")
                                  for m in range(KVC)]
                        sumsq_kv = ss_ps.tile([1, TSH], F32, tag="ssq2")
                        for m in range(KVC):
                            nc.scalar.copy(kv_raw[m][:], kv_ps[m][:])
                            sq = bst.tile([128, TSH], F32R, name="sqk",
                                          tag="sq")
                            nc.vector.tensor_mul(sq[:], kv_raw[m][:],
                                                 kv_raw[m][:])
                            nc.tensor.matmul(sumsq_kv[:], onesc_sb[:], sq[:],
                                             start=(m == 0),
                                             stop=(m == KVC - 1))
                        s_kv = bst.tile([1, TSH], F32, tag="s_small")
                        nc.scalar.activation(s_kv[:], sumsq_kv[:], AF.Sqrt,
                                             bias=eps_sb[:], scale=1.0 / KVR)
                        inv_kv = bst.tile([1, TSH], F32R, tag="inv_small")
                        with nc.allow_low_precision("f32r rounding intended"):
                            nc.vector.reciprocal(inv_kv[:], s_kv[:])
                        binv = ms_ps.tile([128, TSH], F32, tag="msps")
                        nc.tensor.matmul(binv[:], onesr_sb[:], inv_kv[:],
                                         start=True, stop=True)
                        for m in range(KVC):
                            kvn = bst.tile([128, TSH], BF16, name="kvn",
                                           tag="kvn")
                            nc.vector.tensor_mul(kvn[:], kv_raw[m][:], binv[:])
                            nc.sync.dma_start(
                                ag_kv_in[m * 128:(m + 1) * 128, :], kvn[:])

                        # k-rope: rotate + cos/sin (token shard only)
                        krope_raw = bst.tile([ROPE, TSH], F32R, tag="krr")
                        nc.scalar.copy(krope_raw[:], kr_ps[:])
                        rot_ps = ms_ps.tile([ROPE, TSH], F32, tag="msps")
                        nc.tensor.matmul(rot_ps[:], r64_sb[:], krope_raw[:],
                                         start=True, stop=True)
                        t1 = bst.tile([ROPE, TSH], F32, tag="ropet1")
                        nc.vector.tensor_mul(t1[:], krope_raw[:], cosk_sb[:])
                        t2 = bst.tile([ROPE, TSH], F32, tag="ropet2")
                        nc.vector.tensor_mul(t2[:], rot_ps[:], sink_sb[:])
                        krn = bst.tile([ROPE, TSH], BF16, tag="krn")
                        nc.vector.tensor_add(krn[:], t1[:], t2[:])
                        nc.sync.dma_start(ag_kv_in[KVR:, :], krn[:])
                        nc.gpsimd.collective_compute(
                            "AllGather", mybir.AluOpType.bypass,
                            replica_groups=[list(range(NCORES))],
                            ins=[ag_kv_in.opt()], outs=[ag_kv_out.opt()])

                    # CC-stream friendly order: q0, q1, kv, q2
                    q_group(0)
                    q_group(1)
                    kv_down()
                    q_group(2)

            # ============ phase C: up-projections (head-parallel) ==========
            with tc.tile_pool(name="kn_sb", bufs=1) as kn_pool, \
                 tc.tile_pool(name="v_sb", bufs=1) as v_pool, \
                 tc.tile_pool(name="qt_sb", bufs=1) as qt_pool, \
                 tc.tile_pool(name="kr_sb", bufs=1) as kr_pool, \
                 tc.tile_pool(name="wo_sb", bufs=1) as wo_pool:

                knT = kn_pool.tile([128, HPC, TC, 512], BF16)   # 16 KB/part
                v_sb = v_pool.tile([128, TSH // 128 * NCORES, HPC * VHD], BF16)
                qT = qt_pool.tile([128, 3, TC, 512], BF16)      # 24 KB/part
                # k_rope^T doubled into both partition halves so that the
                # rope score matmul's lhsT base_partition matches q's half
                krT = kr_pool.tile([2 * ROPE, TC, 512], BF16)
                nc.sync.dma_start(
                    krT[0:ROPE, :, :],
                    ag_kv_out[:, KVR:, :].transpose([1, 0, 2]))
                nc.sync.dma_start(
                    krT[ROPE:, :, :],
                    ag_kv_out[:, KVR:, :].transpose([1, 0, 2]))
                # Wo preload, head-0 half only (finishes during attention;
                # 32 KB/part bf16). Odd (head-1) chunks stream in phase E.
                wo_sbuf = wo_pool.tile([128, KCO // 2, HID], BF16)

                with tc.tile_pool(name="kvn_sb", bufs=8) as kvn_pool, \
                     tc.tile_pool(name="upw", bufs=1) as upw, \
                     tc.tile_pool(name="up_ps", bufs=6, space="PSUM") as up_ps:
                    wkn_sb = upw.tile([128, KVC, HPC * NOPE], BF16)
                    for k in range(KVC):
                        nc.scalar.dma_start(wkn_sb[:, k, :], wkvb_kn[k])
                    wv_sb = upw.tile([128, KVC, HPC * VHD], BF16)
                    for k in range(KVC):
                        nc.scalar.dma_start(wv_sb[:, k, :], wkvb_v[k])

                    # K_nope^T and V, streaming kv_norm^T tiles from the AG
                    for tcb in range(TC):
                        rh = []
                        for k in range(KVC):
                            rt = kvn_pool.tile([128, 512], BF16, name="kvn_t",
                                               tag="kvn_t")
                            nc.sync.dma_start(
                                rt[:],
                                ag_kv_out[tcb, k * 128:(k + 1) * 128, :])
                            rh.append(rt)
                        psn = [up_ps.tile([128, 512], F32, name=f"knps{hl}",
                                          tag="upps") for hl in range(HPC)]
                        for k in range(KVC):
                            for hl in range(HPC):
                                nc.tensor.matmul(
                                    psn[hl][:],
                                    wkn_sb[:, k, hl * NOPE:(hl + 1) * NOPE],
                                    rh[k][:],
                                    start=(k == 0), stop=(k == KVC - 1))
                        for hl in range(HPC):
                            nc.scalar.copy(knT[:, hl, tcb, :], psn[hl][:])
                        psv = [up_ps.tile([128, HPC * VHD], F32,
                                          name=f"vps{j}", tag="upps")
                               for j in range(4)]
                        for k in range(KVC):
                            for j in range(4):
                                nc.tensor.matmul(
                                    psv[j][:],
                                    rh[k][:, j * 128:(j + 1) * 128],
                                    wv_sb[:, k, :],
                                    start=(k == 0), stop=(k == KVC - 1))
                        for j in range(4):
                            nc.vector.tensor_copy(v_sb[:, tcb * 4 + j, :],
                                                  psv[j][:])

                # Q^T (3 chunks: nope h0, nope h1, rope doubled), deferred
                # RMS normalize folded into the PSUM eviction, rope applied
                # per token-chunk right after.
                with tc.tile_pool(name="agq_sb", bufs=26) as agq_pool, \
                     tc.tile_pool(name="qw_sb", bufs=1) as qw_pool, \
                     tc.tile_pool(name="rope_sb", bufs=2) as rope_pool, \
                     tc.tile_pool(name="ropest", bufs=2) as ropest, \
                     tc.tile_pool(name="qt_ps", bufs=4, space="PSUM") as qt_ps, \
                     tc.tile_pool(name="rr_ps", bufs=2, space="PSUM") as rr_ps, \
                     tc.tile_pool(name="bq_ps", bufs=2, space="PSUM") as bq_ps:
                    wqb_sb = qw_pool.tile([128, QRC, HPC * QKD], BF16)
                    for k in range(QRC):
                        nc.scalar.dma_start(wqb_sb[:, k, :], wqb[k])
                    iv_hi_sb = qw_pool.tile([1, TC, 512], BF16)
                    nc.sync.dma_start(
                        iv_hi_sb[:],
                        ag_q_out[2][:, QR // 3:QR // 3 + 1, :]
                        .transpose([1, 0, 2]))
                    iv_lo_sb = qw_pool.tile([1, TC, 512], BF16)
                    nc.sync.dma_start(
                        iv_lo_sb[:],
                        ag_q_out[2][:, QR // 3 + 1:, :].transpose([1, 0, 2]))

                    def agq_load(tcb):
                        tiles = []
                        for k in range(QRC):
                            rh16 = agq_pool.tile([128, 512], BF16,
                                                 name="agq16", tag="agq16")
                            nc.sync.dma_start(
                                rh16[:],
                                ag_q_out[k // 4][tcb,
                                                 (k % 4) * 128:
                                                 (k % 4 + 1) * 128,
                                                 :])
                            tiles.append(rh16)
                        return tiles

                    nxt = agq_load(0)
                    for tcb in range(TC):
                        cur = nxt
                        if tcb + 1 < TC:
                            nxt = agq_load(tcb + 1)
                        ps = [qt_ps.tile([128, 512], F32, name=f"qtps{m}",
                                         tag="qtps") for m in range(3)]
                        for k in range(QRC):
                            for m in range(3):
                                nc.tensor.matmul(
                                    ps[m][:],
                                    wqb_sb[:, k, m * 128:(m + 1) * 128],
                                    cur[k][:],
                                    start=(k == 0), stop=(k == QRC - 1))
                        # broadcast 1/rms across partitions (hi+lo halves),
                        # then evict with the normalize multiply
                        biq = bq_ps.tile([128, 512], F32, name="biq", tag="biq")
                        nc.tensor.matmul(biq[:], onesr_b[:],
                                         iv_hi_sb[:, tcb, :],
                                         start=True, stop=False)
                        nc.tensor.matmul(biq[:], onesr_b[:],
                                         iv_lo_sb[:, tcb, :],
                                         start=False, stop=True)
                        biq_sb = ropest.tile([128, 512], F32, name="biq_sb",
                                             tag="biq_sb")
                        nc.scalar.copy(biq_sb[:], biq[:])
                        for m in range(2):
                            nc.vector.tensor_mul(qT[:, m, tcb, :], ps[m][:],
                                                 biq_sb[:])
                        # q-rope on chunk m=2 (both heads doubled)
                        qr2 = ropest.tile([128, 512], F32R, name="qr2",
                                          tag="qr2")
                        nc.vector.tensor_mul(qr2[:], ps[2][:], biq_sb[:])
                        cosq_t = rope_pool.tile([128, 512], F32, name="cosq_t",
                                                tag="cosq_t")
                        nc.sync.dma_start(cosq_t[:],
                                          cos_q[:, tcb * 512:(tcb + 1) * 512])
                        sinq_t = rope_pool.tile([128, 512], F32, name="sinq_t",
                                                tag="sinq_t")
                        nc.sync.dma_start(sinq_t[:],
                                          sin_q[:, tcb * 512:(tcb + 1) * 512])
                        rps = rr_ps.tile([128, 512], F32, tag="rrps")
                        nc.tensor.matmul(rps[:], r128_sb[:], qr2[:],
                                         start=True, stop=True)
                        t1 = ropest.tile([128, 512], F32, name="rt1", tag="rt1")
                        nc.vector.tensor_mul(t1[:], qr2[:], cosq_t[:])
                        t2 = ropest.tile([128, 512], F32, name="rt2", tag="rt2")
                        nc.vector.tensor_mul(t2[:], rps[:], sinq_t[:])
                        nc.vector.tensor_add(qT[:, 2, tcb, :], t1[:], t2[:])

                # ============ phase D: causal attention =================
                with tc.tile_pool(name="ao_sb", bufs=1) as ao_pool:
                  # split even/odd so phase E's even-kc reads don't wait on
                  # the second AllToAll (tile-granularity dep tracking)
                  aosb_ev = ao_pool.tile([128, KCO // 2, 512], BF16)
                  aosb_od = ao_pool.tile([128, KCO // 2, 512], BF16)
                  # kick off the Wo even-kc preload now; lands during attention
                  for i in range(KCO // 2):
                      nc.sync.dma_start(wo_sbuf[:, i, :], wo[2 * i])
                  with tc.tile_pool(name="pt_sb", bufs=6) as pt_pool, \
                     tc.tile_pool(name="pa_sb", bufs=2) as pa_pool, \
                     tc.tile_pool(name="att_st", bufs=2) as att_st, \
                     tc.tile_pool(name="st_ps", bufs=4, space="PSUM") as st_ps, \
                     tc.tile_pool(name="l_ps", bufs=1, space="PSUM") as l_ps, \
                     tc.tile_pool(name="o_ps", bufs=2, space="PSUM") as o_ps, \
                     tc.tile_pool(name="bi_ps", bufs=1, space="PSUM") as bi_ps:
                    for hl in range(HPC):
                        for b in range(B):
                            for qc in range(SB):
                                tcq = b * SB + qc
                                nkb = 4 * qc + 4
                                op = o_ps.tile([128, 512], F32, name="op",
                                               tag="op")
                                pacc = pa_pool.tile([128, 512], F32R,
                                                    name="pacc", tag="pacc")
                                # software-pipelined: PE issues st(kb) before
                                # op(kb-1) so it never stalls on exp(kb-1)
                                pend = None   # (pt, colstart, kb)
                                for kb in range(nkb):
                                    tck = b * SB + kb // 4
                                    co = (kb % 4) * 128
                                    diag = kb >= 4 * qc
                                    r = kb - 4 * qc
                                    a = 128 * r if diag else 0
                                    st = st_ps.tile([128, 512], F32,
                                                    name="st", tag="st")
                                    nc.tensor.matmul(
                                        st[:, a:],
                                        knT[:, hl, tck, co:co + 128],
                                        qT[:, hl, tcq, a:],
                                        start=True, stop=False)
                                    nc.tensor.matmul(
                                        st[:, a:],
                                        krT[hl * ROPE:(hl + 1) * ROPE,
                                            tck, co:co + 128],
                                        qT[hl * ROPE:(hl + 1) * ROPE, 2,
                                           tcq, a:],
                                        start=False, stop=True)
                                    if pend is not None:
                                        ppt, pa, pkb = pend
                                        nc.tensor.matmul(
                                            op[:, pa:],
                                            v_sb[:, b * NKB + pkb,
                                                 hl * VHD:(hl + 1) * VHD],
                                            ppt[:, pa:],
                                            start=(pkb == 0), stop=False)
                                    pt = pt_pool.tile([128, 512], BF16,
                                                      name="pt", tag="pt")
                                    if diag:
                                        # unmasked tail + masked 128-col tri
                                        if r < 3:
                                            nc.scalar.activation(
                                                pt[:, a + 128:], st[:, a + 128:],
                                                AF.Exp, scale=SCALE)
                                        pte = pt_pool.tile([128, 128], BF16,
                                                           name="pte",
                                                           tag="pte", bufs=2)
                                        nc.scalar.activation(
                                            pte[:], st[:, a:a + 128],
                                            AF.Exp, scale=SCALE)
                                        nc.vector.tensor_mul(
                                            pt[:, a:a + 128], pte[:],
                                            mask_sb[:])
                                    else:
                                        nc.scalar.activation(pt[:], st[:],
                                                             AF.Exp,
                                                             scale=SCALE)
                                    if kb == 0:
                                        nc.vector.tensor_copy(pacc[:], pt[:])
                                    else:
                                        nc.vector.tensor_add(
                                            pacc[:, a:], pacc[:, a:],
                                            pt[:, a:])
                                    pend = (pt, a, kb)
                                ppt, pa, pkb = pend
                                nc.tensor.matmul(
                                    op[:, pa:],
                                    v_sb[:, b * NKB + pkb,
                                         hl * VHD:(hl + 1) * VHD],
                                    ppt[:, pa:],
                                    start=False, stop=True)
                                lp = l_ps.tile([1, 512], F32, name="lp",
                                               tag="lp")
                                nc.tensor.matmul(lp[:], onesc_sb[:], pacc[:],
                                                 start=True, stop=True)
                                lps = att_st.tile([1, 512], F32R, name="lps",
                                                  tag="lps")
                                nc.scalar.copy(lps[:], lp[:])
                                bi = bi_ps.tile([128, 512], F32, name="bi",
                                                tag="bi")
                                nc.tensor.matmul(bi[:], onesr_sb[:], lps[:],
                                                 start=True, stop=True)
                                biR = att_st.tile([128, 512], F32R,
                                                  name="biR", tag="biR")
                                with nc.allow_low_precision("f32r intended"):
                                    nc.vector.reciprocal(biR[:], bi[:])
                                att = att_st.tile([128, 512], BF16, name="att",
                                                  tag="att")
                                nc.vector.tensor_mul(att[:], op[:], biR[:])
                                nc.sync.dma_start(
                                    a2a_in[hl][tcq, :, :], att[:])
                        nc.gpsimd.collective_compute(
                            "AllToAll", mybir.AluOpType.bypass,
                            replica_groups=[list(range(NCORES))],
                            ins=[a2a_in[hl].opt()], outs=[a2a_out[hl].opt()])
                        aot = aosb_ev if hl == 0 else aosb_od
                        for i in range(NCORES):
                            nc.sync.dma_start(aot[:, i, :], a2a_out[hl][i])

                  # ============ phase E: o_proj (token-parallel) ===========
                  with tc.tile_pool(name="yo_sb", bufs=3) as yo_pool, \
                       tc.tile_pool(name="wo_st", bufs=6) as wo_st, \
                       tc.tile_pool(name="op_ps", bufs=8, space="PSUM") as op_ps:
                    # accumulate kc chunks head-0-first (preloaded in SBUF) so
                    # o_proj can start while the second AllToAll is in flight;
                    # head-1 (odd kc) weights stream from DRAM.
                    kc_order = [2 * i for i in range(NCORES)] + \
                               [2 * i + 1 for i in range(NCORES)]
                    for n in range(HID // 512):
                        pso = [op_ps.tile([128, 512], F32, name=f"pso{mt}",
                                          tag="pso") for mt in range(4)]
                        for ki, kc in enumerate(kc_order):
                            if kc % 2 == 0:
                                wot = wo_sbuf[:, kc // 2,
                                              n * 512:(n + 1) * 512]
                                aot = aosb_ev[:, kc // 2, :]
                            else:
                                wot_t = wo_st.tile([128, 512], BF16,
                                                   name="wot", tag="wot")
                                nc.scalar.dma_start(
                                    wot_t[:], wo[kc, :, n * 512:(n + 1) * 512])
                                wot = wot_t[:]
                                aot = aosb_od[:, kc // 2, :]
                            for mt in range(4):
                                nc.tensor.matmul(
                                    pso[mt][:],
                                    aot[:, mt * 128:(mt + 1) * 128],
                                    wot,
                                    start=(ki == 0), stop=(ki == KCO - 1))
                        for mt in range(4):
                            yt = yo_pool.tile([128, 512], F32, name="yt",
                                              tag="yt")
                            nc.scalar.copy(yt[:], pso[mt][:])
                            nc.sync.dma_start(
                                y[mt * 128:(mt + 1) * 128,
                                  n * 512:(n + 1) * 512], yt[:])

    nc.compile()
    return nc


def _rot_lhsT(n):
    """lhsT for the interleaved rotate-half as a matmul: out = R @ x,
    R[2i, 2i+1] = -1, R[2i+1, 2i] = +1; matmul computes lhsT.T @ rhs."""
    R = np.zeros((n, n), dtype=np.float32)
    for i in range(n // 2):
        R[2 * i, 2 * i + 1] = -1.0
        R[2 * i + 1, 2 * i] = 1.0
    return np.ascontiguousarray(R.T)


def _prep_inputs(inputs):
    """Host-side sharding/reordering. Returns in_maps (list of 8 dicts)."""
    import ml_dtypes
    BF = ml_dtypes.bfloat16

    hs = np.asarray(inputs["hidden_states"], dtype=np.float32).reshape(T, IN)
    hsT = np.ascontiguousarray(hs.T)                  # [IN, T] host transpose
    Wq_a = np.asarray(inputs["Wq_a"], dtype=np.float32)
    q_a_ln = np.asarray(inputs["q_a_ln"], dtype=np.float32)
    Wq_b = np.asarray(inputs["Wq_b"], dtype=np.float32) * q_a_ln[:, None]
    Wkv_a = np.asarray(inputs["Wkv_a"], dtype=np.float32)
    kv_a_ln = np.asarray(inputs["kv_a_ln"], dtype=np.float32)
    Wkv_b = np.asarray(inputs["Wkv_b"], dtype=np.float32) * kv_a_ln[:, None]
    Wo = np.asarray(inputs["Wo"], dtype=np.float32)
    pos = np.asarray(inputs["position_ids"]).astype(np.float64)   # [B, S]

    # rope tables (doubled pairs): cos[2i] = cos[2i+1] = cos(pos * invf_i)
    invf = 1.0 / (THETA ** (np.arange(0, ROPE, 2, dtype=np.float64) / ROPE))
    fr = pos[..., None] * invf                       # [B, S, 32]
    cosd = np.repeat(np.cos(fr), 2, axis=-1).astype(np.float32)  # [B, S, 64]
    sind = np.repeat(np.sin(fr), 2, axis=-1).astype(np.float32)
    cosT = np.ascontiguousarray(cosd.reshape(T, ROPE).T)   # [64, T]
    sinT = np.ascontiguousarray(sind.reshape(T, ROPE).T)
    cos_q = np.concatenate([cosT, cosT], axis=0)           # [128, T]
    sin_q = np.concatenate([sinT, sinT], axis=0)

    # causal multiplicative 0/1 triangle for the 128x128 diagonal sub-block
    kl = np.arange(128)[:, None]
    ql = np.arange(128)[None, :]
    mask01 = np.where(kl > ql, 0.0, 1.0).astype(BF)

    onesc = np.ones((128, 1), dtype=np.float32)
    onesr = np.ones((1, 128), dtype=np.float32)
    onesrb = np.ones((1, 128), dtype=BF)

    wqa_r = np.ascontiguousarray(
        Wq_a.reshape(IN // 128, 128, QR).astype(BF))
    wkva_r = np.ascontiguousarray(
        Wkv_a.reshape(IN // 128, 128, KVR + ROPE).astype(BF))
    wo_r = np.ascontiguousarray(
        Wo.reshape(H * VHD // 128, 128, HID).astype(BF))

    Wq_b_h = Wq_b.reshape(QR, H, QKD)
    Wkv_b_h = Wkv_b.reshape(KVR, H, NOPE + VHD)

    in_maps = []
    for c in range(NCORES):
        h0, h1 = HPC * c, HPC * c + 1
        bc = c // (NCORES // B)
        s0 = (c % (NCORES // B)) * TSH
        # reorder q_b cols: [nope_h0 | nope_h1 | rope_h0 ; rope_h1]
        wqb_s = np.concatenate([
            Wq_b_h[:, h0, :NOPE], Wq_b_h[:, h1, :NOPE],
            Wq_b_h[:, h0, NOPE:], Wq_b_h[:, h1, NOPE:]], axis=1)
        wqb_s = np.ascontiguousarray(
            wqb_s.reshape(QR // 128, 128, HPC * QKD).astype(BF))
        wkvb_kn_s = np.ascontiguousarray(
            np.concatenate([Wkv_b_h[:, h0, :NOPE], Wkv_b_h[:, h1, :NOPE]],
                           axis=1).reshape(KVR // 128, 128,
                                           HPC * NOPE).astype(BF))
        wkvb_v_s = np.ascontiguousarray(
            np.concatenate([Wkv_b_h[:, h0, NOPE:], Wkv_b_h[:, h1, NOPE:]],
                           axis=1).reshape(KVR // 128, 128,
                                           HPC * VHD).astype(BF))
        tok0 = c * TSH
        hidt_s = np.ascontiguousarray(
            hsT[:, tok0:tok0 + TSH].reshape(IN // 128, 128, TSH).astype(BF))
        in_maps.append({
            "hidt": hidt_s,
            "wqa": wqa_r, "wqb": wqb_s, "wkva": wkva_r,
            "wkvb_kn": wkvb_kn_s, "wkvb_v": wkvb_v_s, "wo": wo_r,
            "cos_k": np.ascontiguousarray(
                cosT[:, bc * S + s0: bc * S + s0 + TSH]),
            "sin_k": np.ascontiguousarray(
                sinT[:, bc * S + s0: bc * S + s0 + TSH]),
            "cos_q": cos_q, "sin_q": sin_q,
            "mask01": mask01,
            "r128": np.ascontiguousarray(
                np.block([[_rot_lhsT(ROPE), np.zeros((ROPE, ROPE), np.float32)],
                          [np.zeros((ROPE, ROPE), np.float32), _rot_lhsT(ROPE)]])),
            "r64": _rot_lhsT(ROPE),
            "onesc": onesc, "onesr": onesr, "onesrb": onesrb,
        })
    return in_maps


def kernel(**inputs) -> np.ndarray:
    from concourse.bass_utils import run_bass_kernel_spmd

    if "nc" not in _cache:
        _cache["nc"] = _build()
    nc = _cache["nc"]
    in_maps = _prep_inputs(inputs)
    res = run_bass_kernel_spmd(nc, in_maps, core_ids=list(range(NCORES)))
    out = np.concatenate([res.results[c]["y"] for c in range(NCORES)], axis=0)
    return np.ascontiguousarray(out.reshape(B, S, HID))


if __name__ == "__main__":
    rng = np.random.default_rng(0)
    ins = {
        "hidden_states": rng.standard_normal((B, S, IN), dtype=np.float32),
        "Wq_a": rng.standard_normal((IN, QR), dtype=np.float32) * IN ** -0.5,
        "q_a_ln": np.ones(QR, np.float32),
        "Wq_b": rng.standard_normal((QR, H * QKD), dtype=np.float32) * QR ** -0.5,
        "Wkv_a": rng.standard_normal((IN, KVR + ROPE), dtype=np.float32) * IN ** -0.5,
        "kv_a_ln": np.ones(KVR, np.float32),
        "Wkv_b": rng.standard_normal((KVR, H * (NOPE + VHD)), dtype=np.float32) * KVR ** -0.5,
        "Wo": rng.standard_normal((H * VHD, HID), dtype=np.float32) * (H * VHD) ** -0.5,
        "position_ids": np.tile(np.arange(S, dtype=np.int32)[None], (B, 1)),
    }
    out = kernel(**ins)
    print("kernel ran, out shape", out.shape, "absmax", np.abs(out).max())
